# revision 51
# baseline (speedup 1.0000x reference)
"""Causal MHA (B=2, L=2048, D=1024, 16 heads, RoPE) on 8 Trainium2 NeuronCores.

Tensor-parallel over heads (2 heads/core), engine-overlap structure:
 - hd dims of q/k are permuted (pairs (i, i+32) interleaved) so the RoPE
   rotate-half swap is an adjacent-partition-pair stream_shuffle on DVE;
   scores are invariant under a shared q/k hd permutation.  RoPE reads the
   projection PSUM directly (shuffle + 2 muls + 2 adds, no scalar copy);
   the Scalar engine runs exp only.
 - Attention processes both heads per pass: the two heads' score matmuls run
   concurrently in PE row-groups 0-1 / 2-3 (tile_position auto-derived from
   base partitions), one exp per [128, 1024] (h0|h1) PSUM tile.  PV uses a
   65-col stationary [V | ones] so the softmax denominator falls out of the
   same matmul stream.
 - V is projected 512-wide as V^T (weight-stationary) and flipped token-major
   with XBAR transpose DMAs through a dense staging tile - 1/4 the PE matmuls
   of a direct token-major projection.
 - Projection chunks of later batches ride as PE fillers inside earlier
   attention passes, so proj work overlaps Scalar exp work and the PSUM
   tile rotation never stalls at pass boundaries.
 - The AllToAll is split by batch; cc#1 fires right after pass(0,1) and its
   cross-core (launch-skew) wait hides under the two batch-1 passes, which
   are pure SBUF compute (DMA engines freeze during a collective; the
   normalize outputs buffer in 10 ht tiles until the DMA queue thaws).
   Batch-0's output projection overlaps cc#2's flight.
 - x is DMA'd in [128, 1024] column chunks interleaved with the qkv weight
   tiles so the first projection matmul starts ~6us in.
"""

import numpy as np

import concourse.bass as bass
import concourse.mybir as mybir
import concourse.tile as tile
from concourse import bacc
from concourse.bass_utils import run_bass_kernel_spmd

B, L, D, NH, HD = 2, 2048, 1024, 16, 64
ROPE_BASE = 10000.0
N_CORES = 8
HPC = NH // N_CORES          # heads per core = 2
M = B * L                    # 4096 tokens
KT = D // 128                # 8 contraction tiles
QB = 512                     # q chunk in attention
KB = 128                     # k block in attention
NQC = L // QB                # 4

fp16 = mybir.dt.float16
fp32 = mybir.dt.float32

# adjacent-pair swap within each 32-partition quadrant (rotate-half partner)
SWAP_MASK = [i ^ 1 for i in range(32)]

_NC = None


def _build_nc():
    nc = bacc.Bacc("TRN2", target_bir_lowering=False, debug=False,
                   num_devices=N_CORES)

    xT = nc.dram_tensor("xT", [D, M], fp16, kind="ExternalInput").ap()
    wqkvT = nc.dram_tensor("wqkvT", [D, 384], fp16, kind="ExternalInput").ap()
    woutT = nc.dram_tensor("woutT", [D, D], fp16, kind="ExternalInput").ap()
    cosT = nc.dram_tensor("cosT", [128, L], fp32, kind="ExternalInput").ap()
    sinT = nc.dram_tensor("sinT", [128, L], fp32, kind="ExternalInput").ap()
    out = nc.dram_tensor("out", [D, QB], fp32, kind="ExternalOutput").ap()

    cc_in = [nc.dram_tensor(f"cc_in{n}", [D, 256], fp16) for n in range(2)]
    cc_out = [nc.dram_tensor(f"cc_out{n}", [D, 256], fp16) for n in range(2)]

    with tile.TileContext(nc) as tc:
        with tc.tile_pool(name="persist", bufs=1) as per, \
             tc.tile_pool(name="weights", bufs=1) as wp:
            wq = [wp.tile([128, 384], fp16, tag=f"wq{k}", name=f"wq{k}")
                  for k in range(KT)]
            wo = [wp.tile([128, D], fp16, tag=f"wo{k}", name=f"wo{k}")
                  for k in range(KT)]

            cos_t = per.tile([128, L], fp32, tag="cos")
            sin_t = per.tile([128, L], fp32, tag="sin")

            # doubled triangular mask (h0|h1): keep where q-col >= k-row
            mask01 = per.tile([128, 1024], fp16, tag="mask01")
            nc.gpsimd.memset(mask01[:], 1.0)
            for h in range(2):
                nc.gpsimd.affine_select(
                    out=mask01[:, h * 512:(h + 1) * 512],
                    in_=mask01[:, h * 512:(h + 1) * 512],
                    compare_op=mybir.AluOpType.is_ge,
                    fill=0.0, base=0, channel_multiplier=-1,
                    pattern=[[1, QB]],
                )

            # Q^T/K^T per batch (hd-permuted): rows 0-63 head0, 64-127 head1;
            # cols 0:L = Q, L:2L = K
            qku = [per.tile([128, 2 * L], fp16, tag=f"qku{b}", name=f"qku{b}")
                   for b in range(B)]
            # V' per batch: per k-tile 130 cols = [v_h0(64) | 1 | v_h1(64) | 1]
            vt = [per.tile([128, (L // 128) * 130], fp16, tag=f"vt{b}",
                           name=f"vt{b}") for b in range(B)]
            for b in range(B):
                nc.gpsimd.memset(vt[b][:], 1.0)

            # x^T tiles, chunk-loaded in consumption order (b-major).
            # cos/sin after the first x chunk (rope gate), wo last (tail-only).
            xt = [per.tile([128, M], fp16, tag=f"xt{k}", name=f"xt{k}")
                  for k in range(KT)]

            def load_x(b, mcp):
                lo = b * 2048 + mcp * 1024
                for k in range(KT):
                    nc.sync.dma_start(xt[k][:, lo:lo + 1024],
                                      xT[k * 128:(k + 1) * 128, lo:lo + 1024])

            # first k-tile's weight + data pairs so matmul 0 starts ASAP
            for k in range(KT):
                nc.sync.dma_start(wq[k][:], wqkvT[k * 128:(k + 1) * 128, :])
                nc.sync.dma_start(xt[k][:, 0:512],
                                  xT[k * 128:(k + 1) * 128, 0:512])
            for k in range(KT):
                nc.sync.dma_start(xt[k][:, 512:1024],
                                  xT[k * 128:(k + 1) * 128, 512:1024])
            nc.sync.dma_start(cos_t[:], cosT[:])
            nc.sync.dma_start(sin_t[:], sinT[:])
            load_x(0, 1)
            load_x(1, 0)
            load_x(1, 1)
            for k in range(KT):
                nc.sync.dma_start(wo[k][:], woutT[k * 128:(k + 1) * 128, :])

            # One PSUM pool: tag "st" [128,1024] x2 (8KB/part) shared by
            # qkv-proj, attention scores and out-proj; tag "hacc" x4 (8KB)
            # = exactly the 16KB/partition of PSUM.
            stps = tc.alloc_tile_pool(name="mm_ps", bufs=2, space="PSUM")
            hps = tc.alloc_tile_pool(name="h_ps", bufs=4, space="PSUM")
            rsb = tc.alloc_tile_pool(name="rope_sb", bufs=3)
            ptp = tc.alloc_tile_pool(name="pt_sb", bufs=6)
            nsb = tc.alloc_tile_pool(name="norm_sb", bufs=2)
            # ht buffers deep enough that normalize never back-pressures on
            # cc_in DMA drain (DMA engines freeze during a collective)
            htp = tc.alloc_tile_pool(name="ht_sb", bufs=10)
            osb = tc.alloc_tile_pool(name="op_sb", bufs=2)
            hfp = tc.alloc_tile_pool(name="htf_sb", bufs=1)

            def mm_psum():
                return stps.tile([128, 1024], fp32, tag="st", name="st")

            def proj_qk_chunk(b, lh, mcb):
                        qkt = mm_psum()
                        qkp = qkt[:, 0:QB]
                        for k in range(KT):
                            nc.tensor.matmul(
                                qkp,
                                wq[k][:, lh * 128:(lh + 1) * 128],
                                xt[k][:, b * L + mcb * QB:
                                      b * L + (mcb + 1) * QB],
                                start=(k == 0), stop=(k == KT - 1))
                        cs = slice(mcb * QB, (mcb + 1) * QB)
                        tmp = rsb.tile([128, QB], fp32, tag="tmp")
                        nc.vector.stream_shuffle(tmp[:], qkp, SWAP_MASK)
                        a16 = rsb.tile([128, QB], fp16, tag="a16")
                        b16 = rsb.tile([128, QB], fp16, tag="b16")
                        nc.vector.tensor_mul(a16[:], qkp, cos_t[:, cs])
                        nc.vector.tensor_mul(b16[:], tmp[:], sin_t[:, cs])
                        # head lh's q rows / k rows within the 128-row block
                        qrows = (slice(0, 64) if lh == 0 else slice(64, 128))
                        krows = (slice(64, 128) if lh == 0 else slice(0, 64))
                        drows = slice(lh * 64, (lh + 1) * 64)
                        bcol = mcb * QB
                        nc.vector.tensor_add(
                            qku[b][drows, bcol:bcol + QB],
                            a16[qrows, :], b16[qrows, :])
                        nc.vector.tensor_add(
                            qku[b][drows, L + bcol:L + bcol + QB],
                            a16[krows, :], b16[krows, :])

            def proj_qk(b):
                for lh in range(HPC):
                    for mcb in range(NQC):
                        proj_qk_chunk(b, lh, mcb)

            def proj_vt_chunk(b, mcb):
                # V^T [128 vrows, 512] via weight-stationary 512-wide matmuls,
                # then PE-transpose each 128-token block into token-major vt.
                vtp = mm_psum()
                vch = vtp[:, 0:QB]
                for k in range(KT):
                    nc.tensor.matmul(
                        vch, wq[k][:, 256:384],
                        xt[k][:, b * L + mcb * QB:b * L + (mcb + 1) * QB],
                        start=(k == 0), stop=(k == KT - 1))
                v16 = rsb.tile([128, QB], fp16, tag="v16")
                nc.vector.tensor_scalar_add(v16[:], vch, 0.0)
                for j in range(4):
                    mt = mcb * 4 + j
                    stg = rsb.tile([128, 128], fp16, tag="vstg")
                    nc.sync.dma_start_transpose(stg[:],
                                                v16[:, j * 128:(j + 1) * 128])
                    dst = vt[b][:, mt * 130:mt * 130 + 130]
                    dst = dst.rearrange("p (g c) -> p g c", g=2)[:, :, 0:64]
                    nc.vector.tensor_scalar_add(
                        dst, stg[:].rearrange("p (g c) -> p g c", g=2), 0.0)

            def pass_(b, pas, fillers=()):
                fillers = list(fillers)
                qcs = (2 * pas, 2 * pas + 1)
                kmax = (qcs[1] + 1) * (QB // KB)
                hacc = {(qc, h): hps.tile([65, QB], fp32, tag="hacc",
                                          name="hacc")
                        for qc in qcs for h in range(2)}

                def normalize(qc, h):
                    ha = hacc[(qc, h)]
                    dsb = nsb.tile([1, QB], fp32, tag="dsb")
                    nc.vector.tensor_scalar_add(dsb[:], ha[64:65, :], 0.0)
                    recip = nsb.tile([1, QB], fp32, tag="recip")
                    nc.vector.reciprocal_approx_fast(recip[:], dsb[:])
                    rb = nsb.tile([64, QB], fp32, tag="rb")
                    nc.gpsimd.partition_broadcast(rb[:], recip[:])
                    ht = htp.tile([64, QB], fp16, tag="ht")
                    nc.vector.tensor_mul(ht[:], ha[0:64, :], rb[:])
                    for half in range(2):
                        cp = qc * 2 + half
                        nc.sync.dma_start(
                            cc_in[b].ap()[cp * 128 + h * 64:
                                          cp * 128 + h * 64 + 64, :],
                            ht[:, half * 256:half * 256 + 256])

                def emit_pv(slot):
                    ki, qc, off, w, pt = slot
                    last = (ki == (qc + 1) * (QB // KB) - 1)
                    for h in range(2):
                        vsl = vt[b][:, ki * 130 + h * 65:ki * 130 + h * 65 + 65]
                        nc.tensor.matmul(
                            hacc[(qc, h)][:, off:off + w], vsl,
                            pt[:, h * 512:h * 512 + w],
                            start=(ki == 0), stop=last)
                    if last:
                        normalize(qc, 0)
                        normalize(qc, 1)

                pend = []
                n_fill = len(fillers)
                for ki in range(kmax):
                    # front-load fillers: proj work covers the scalar-exp lag
                    # carried over from the previous pass's last score tiles
                    keep = max(0, n_fill - 2 * (ki + 1))
                    while len(fillers) > keep:
                        fillers.pop(0)()
                    qlo = max(qcs[0], ki // (QB // KB))
                    for qc in range(qlo, qcs[1] + 1):
                        diag = (qc == ki // (QB // KB))
                        off = (ki % (QB // KB)) * KB if diag else 0
                        w = QB - off
                        st = mm_psum()
                        for h in range(2):
                            nc.tensor.matmul(
                                st[:, h * 512:h * 512 + w],
                                qku[b][h * 64:(h + 1) * 64,
                                       L + ki * KB:L + (ki + 1) * KB],
                                qku[b][h * 64:(h + 1) * 64,
                                       qc * QB + off:(qc + 1) * QB],
                                start=True, stop=True)
                        pt = ptp.tile([128, 1024], fp16, tag="pt", name="pt")
                        nc.scalar.activation(
                            pt[:], st[:], mybir.ActivationFunctionType.Exp)
                        if diag:
                            ptv = pt[:].rearrange(
                                "p (g c) -> p g c", g=2)[:, :, 0:w]
                            mkv = mask01[:].rearrange(
                                "p (g c) -> p g c", g=2)[:, :, 0:w]
                            nc.vector.tensor_mul(ptv, ptv, mkv)
                        pend.append((ki, qc, off, w, pt))
                        if len(pend) >= 2:
                            emit_pv(pend.pop(0))
                while pend:
                    emit_pv(pend.pop(0))

            # separate tiles per collective half: a shared tile would give
            # whole-tile deps and serialize outproj(0) behind cc#2's loads
            htf = [[hfp.tile([128, 256], fp16, tag=f"htf{n}_{k}",
                             name=f"htf{n}_{k}") for k in range(KT)]
                   for n in range(2)]

            def htf_load(n):
                for k in range(KT):
                    nc.sync.dma_start(
                        htf[n][k][:],
                        cc_out[n].ap()[k * 128:(k + 1) * 128, :])

            def outproj(n):
                # half-width sweep per collective so batch-0's projection
                # overlaps cc#2's flight
                for eb in range(KT):
                    opt_ = mm_psum()
                    op = opt_[:, 0:256]
                    for k in range(KT):
                        nc.tensor.matmul(
                            op, wo[k][:, eb * 128:(eb + 1) * 128],
                            htf[n][k][:],
                            start=(k == 0), stop=(k == KT - 1))
                    ot = osb.tile([128, 256], fp32, tag="ot")
                    nc.scalar.copy(ot[:], op)
                    nc.sync.dma_start(
                        out[eb * 128:(eb + 1) * 128, n * 256:(n + 1) * 256],
                        ot[:])

            import functools

            def fill_set(b, lo_half):
                # proj chunks needed by the NEXT pass: q/k chunks mcb pair +
                # the matching V^T chunks
                mcbs = (0, 1) if lo_half else (2, 3)
                f = [functools.partial(proj_qk_chunk, b, lh, mcb)
                     for lh in range(HPC) for mcb in mcbs]
                f += [functools.partial(proj_vt_chunk, b, mcb) for mcb in mcbs]
                return f

            # upfront: exactly what pass(0,0) needs
            for fn in fill_set(0, True):
                fn()
            # Batch-split collectives: cc#1 carries ALL of batch 0 and fires
            # right after pass(0,1), hiding its cross-core skew wait under
            # the two batch-1 attention passes (pure SBUF compute; their
            # cc_in writes buffer in ht tiles until DMA unfreezes).
            pass_(0, 0, fillers=fill_set(0, False))
            pass_(0, 1, fillers=fill_set(1, True) + fill_set(1, False))
            nc.gpsimd.collective_compute(
                "AllToAll", mybir.AluOpType.bypass,
                replica_groups=[list(range(N_CORES))],
                ins=[cc_in[0].ap().opt()], outs=[cc_out[0].ap().opt()],
            )
            pass_(1, 0)
            htf_load(0)      # cc_out0 ready mid-pass; prefetch for outproj(0)
            pass_(1, 1)
            nc.gpsimd.collective_compute(
                "AllToAll", mybir.AluOpType.bypass,
                replica_groups=[list(range(N_CORES))],
                ins=[cc_in[1].ap().opt()], outs=[cc_out[1].ap().opt()],
            )
            outproj(0)       # batch-0 projection overlaps cc#2 flight
            htf_load(1)
            outproj(1)

            for pool in (hfp, osb, htp, nsb, ptp, rsb, hps, stps):
                pool.release()

    nc.compile()
    return nc


def _host_inputs(x, Wqkv, Wout):
    """Build the 8 per-core input maps."""
    x = np.asarray(x, dtype=np.float32)
    Wqkv = np.asarray(Wqkv, dtype=np.float32)
    Wout = np.asarray(Wout, dtype=np.float32)

    xT = np.ascontiguousarray(x.reshape(M, D).T).astype(np.float16)
    woutT = np.ascontiguousarray(Wout.T).astype(np.float16)

    scale = HD ** -0.5
    # hd permutation: rotate-half partners (i, i+32) -> rows (2i, 2i+1)
    perm = np.empty(64, dtype=np.int64)
    perm[0::2] = np.arange(32)
    perm[1::2] = np.arange(32) + 32

    inv = ROPE_BASE ** (-np.arange(32, dtype=np.float64) / 32.0)
    l = np.arange(L, dtype=np.float64)
    ang = l[None, :] * inv[:, None]                      # [32, L]
    cos64 = np.repeat(np.cos(ang), 2, axis=0)            # rows (2i,2i+1)=freq i
    sin64 = np.empty((64, L))
    sin64[0::2] = -np.sin(ang)                           # out_t1 = t1 c - t2 s
    sin64[1::2] = np.sin(ang)                            # out_t2 = t2 c + t1 s
    cosT = np.tile(cos64, (2, 1)).astype(np.float32)     # [128, L]
    sinT = np.tile(sin64, (2, 1)).astype(np.float32)

    in_maps = []
    for c in range(N_CORES):
        a = HPC * c
        cols = []
        cols.append((Wqkv[HD * a:HD * (a + 1), :] * scale)[perm])      # q_a
        cols.append(Wqkv[D + HD * a:D + HD * (a + 1), :][perm])        # k_a
        cols.append(Wqkv[D + HD * (a + 1):D + HD * (a + 2), :][perm])  # k_a1
        cols.append((Wqkv[HD * (a + 1):HD * (a + 2), :] * scale)[perm])
        cols.append(Wqkv[2 * D + HD * a:2 * D + HD * (a + 1), :])      # v_a
        cols.append(Wqkv[2 * D + HD * (a + 1):2 * D + HD * (a + 2), :])
        wqkvT = np.ascontiguousarray(
            np.concatenate(cols, 0).T).astype(np.float16)
        in_maps.append({"xT": xT, "wqkvT": wqkvT, "woutT": woutT,
                        "cosT": cosT, "sinT": sinT})
    return in_maps


def kernel(x, Wqkv, Wout, _trace=False):
    global _NC
    if _NC is None:
        _NC = _build_nc()
    in_maps = _host_inputs(x, Wqkv, Wout)
    res = run_bass_kernel_spmd(_NC, in_maps, core_ids=list(range(N_CORES)),
                               trace=_trace)
    outT = np.empty((D, M), dtype=np.float32)
    for c in range(N_CORES):
        r = res.results[c]["out"]                        # [D, 512]
        q0, half = c // 2, c % 2
        c0 = q0 * 512 + half * 256                       # batch 0 chunk
        outT[:, c0:c0 + 256] = r[:, 0:256]
        c1 = 2048 + q0 * 512 + half * 256                # batch 1 chunk
        outT[:, c1:c1 + 256] = r[:, 256:512]
    full = outT.T.reshape(B, L, D).astype(np.float32)
    if _trace:
        kernel.last_results = res
    return full
